# revision 1
# baseline (speedup 1.0000x reference)
"""DiffLogicLayer Trainium2 kernel (host-gather sharding + fp16 streaming,
phase-serialized DMA).

Math: for each output neuron o with inputs a = x[:, ia[o]], b = x[:, ib[o]],
the 16 relaxed binary gates are all linear in {1, a, b, a*b}:

    gate_k(a, b) = C[k,0] + C[k,1]*a + C[k,2]*b + C[k,3]*a*b

so with w = softmax(weights[o]) the layer output collapses to

    out[n, o] = W0[o] + W1[o]*a + W2[o]*b + W3[o]*a*b,   W = softmax(weights) @ C

Sharding: tensor-parallel over out_dim (1024 neurons/core). The gather
x[:, idx] is pure data movement, so it is folded into the host-side input
sharding: each core receives its 2048 gathered rows of x^T pre-packed fp16.

Measured on this part: HBM reads alone sustain ~440 GB/s, writes ~360,
mixed ~330. So ALL loads and ALL stores go on the SAME sync-HWDGE ring:
FIFO drain => loads stream solo at read bandwidth, stores (queued behind,
gated on compute sems) drain after. ~19 DMAs avoids completion-semaphore
lane cross-gating (v3 lesson); the 8 lanes are GLOBAL across rings, and
the scalar HWDGE ring's first DMA completes ~5us late (cold start), so
everything stays on the sync ring (v14 lesson). GPSIMD is NOT used for
elementwise work: it shares SBUF ports with DVE and slows concurrent DVE
ops ~4x (v5 lesson). Ops stay full-tile (128, 2048): sliced/strided DVE
APs lose the 2x/4x perf modes (v7 lesson). scalar_tensor_tensor and the
custom DVE ops (AFFINE_*) run 1x-only (no 2x uops) — fusing v+o or u+t
through them is net slower (v13 lesson).

Compute per block: u = W3*a + W2 (ACT), v = W1*a + W0 (DVE tensor_scalar,
4x fp16), t = u*b, o = t + v (DVE tensor_tensor, 2x fp16). The DVE chain
is the critical path (~26us); it starts as soon as g0's completion sem
fires. Softmax+C-fold is fused via stride-0 broadcast APs over a 16KB C
constant so W0..W3 are ready before g0 lands. Block 7 splits a7 (loaded
right after g0; u7/v7 prepped in mid-stream slack) from b7 (loaded last;
only t+o+store remain at the tail).

Output fp16; host concatenates, transposes, casts to fp32. Max rel err vs
fp32 reference ~4e-3 (tolerance 2e-2).
"""

import os
import sys

import numpy as np

sys.path.insert(0, "/opt/trn_rl_repo")

import concourse.bacc as bacc
import concourse.mybir as mybir
from concourse import tile
from concourse.bass import broadcast_tensor_aps
from concourse.bass_utils import run_bass_kernel_spmd

AF = mybir.ActivationFunctionType
ALU = mybir.AluOpType
AX = mybir.AxisListType
F32 = mybir.dt.float32
F16 = mybir.dt.float16

IN_DIM = 8192
OUT_DIM = 8192
BATCH = 2048
N_CORES = 8
OPC = OUT_DIM // N_CORES  # 1024 neurons per core
NBLK = OPC // 128  # 8 partition blocks per core
HB = BATCH // 2

# gate_k = C[k,0] + C[k,1]*a + C[k,2]*b + C[k,3]*ab  (difflogic convention)
_C = np.array(
    [
        [0, 0, 0, 0],  # False
        [0, 0, 0, 1],  # a AND b
        [0, 1, 0, -1],  # a AND NOT b
        [0, 1, 0, 0],  # a
        [0, 0, 1, -1],  # NOT a AND b
        [0, 0, 1, 0],  # b
        [0, 1, 1, -2],  # XOR
        [0, 1, 1, -1],  # OR
        [1, -1, -1, 1],  # NOR
        [1, -1, -1, 2],  # XNOR
        [1, 0, -1, 0],  # NOT b
        [1, 0, -1, 1],  # a OR NOT b
        [1, -1, 0, 0],  # NOT a
        [1, -1, 0, 1],  # NOT a OR b
        [1, 0, 0, -1],  # NAND
        [1, 0, 0, 0],  # True
    ],
    dtype=np.float32,
)

_PROGRAM = None


def _build_program():
    nc = bacc.Bacc("TRN2", target_bir_lowering=False, debug=False)

    wpre = nc.dram_tensor("wpre", (128, NBLK * 16), F32, kind="ExternalInput")
    cb64 = nc.dram_tensor("cb64", (128, 4 * 16), F32, kind="ExternalInput")
    ga7 = nc.dram_tensor("ga7", (128, BATCH), F16, kind="ExternalInput")
    g0h = [
        nc.dram_tensor(f"g0h{h}", (128, BATCH), F16, kind="ExternalInput") for h in range(2)
    ]
    gblk = [
        nc.dram_tensor(f"g{j}", (128, 2 * BATCH), F16, kind="ExternalInput")
        for j in range(1, NBLK - 1)
    ]
    gb7 = nc.dram_tensor("gb7", (128, BATCH), F16, kind="ExternalInput")
    y0h = [
        nc.dram_tensor(f"y0h{h}", (128, HB), F16, kind="ExternalOutput") for h in range(2)
    ]
    ys = [
        nc.dram_tensor(f"y{j}", (128, BATCH), F16, kind="ExternalOutput")
        for j in range(1, NBLK)
    ]

    with tile.TileContext(nc) as tc:
        with (
            tc.tile_pool(name="const", bufs=1) as cpool,
            tc.tile_pool(name="gath", bufs=1) as gpool,
            tc.tile_pool(name="work", bufs=3) as wpool,
            tc.tile_pool(name="outp", bufs=1) as opool,
        ):
            # ---- loads: all on the sync HWDGE ring, in stream order ----
            wpre_t = cpool.tile([128, NBLK * 16], F32)
            nc.sync.dma_start(wpre_t[:, :], wpre[:, :])
            cb64_t = cpool.tile([128, 4 * 16], F32)
            nc.sync.dma_start(cb64_t[:, :], cb64[:, :])
            # block 0 as two 512KB column-half loads: the first half's
            # completion sem fires ~1.3us before a full-1MB g0 would, so the
            # DVE chain starts earlier.
            g0h_t = []
            for h in range(2):
                t = gpool.tile([128, BATCH], F16, name=f"g0h{h}", tag=f"g0h{h}")
                nc.sync.dma_start(t[:, :], g0h[h][:, :])
                g0h_t.append(t)
            # a7 loads after g2 (it is not needed until the u7/v7 prep,
            # which runs after block 3): keeps g1/g2 arriving ~1.2us earlier,
            # closing the measured DVE wait-for-g1 gap.
            g_t = []
            ga7_t = None
            for j in range(1, NBLK - 1):
                t = gpool.tile([128, 2 * BATCH], F16, tag=f"g{j}")
                nc.sync.dma_start(t[:, :], gblk[j - 1][:, :])
                g_t.append(t)
                if j == 2:
                    ga7_t = gpool.tile([128, BATCH], F16, tag="ga7")
                    nc.sync.dma_start(ga7_t[:, :], ga7[:, :])
            gb7_t = gpool.tile([128, BATCH], F16, tag="gb7")
            nc.sync.dma_start(gb7_t[:, :], gb7[:, :])

            # ---- softmax over the 16 gate logits + C-fold, fused ----
            e_t = cpool.tile([128, NBLK * 16], F32)
            nc.scalar.activation(e_t[:, :], wpre_t[:, :], AF.Exp)
            s_t = cpool.tile([128, NBLK], F32)
            nc.vector.tensor_reduce(
                s_t[:, :], e_t[:, :].rearrange("p (j k) -> p j k", k=16), AX.X, op=ALU.add
            )
            r_t = cpool.tile([128, NBLK], F32)
            nc.vector.reciprocal(r_t[:, :], s_t[:, :])
            # tmp[p, c, j, k] = e[p, j, k] * C[k, c]  (e bcast over c, C over j)
            tmp_t = cpool.tile([128, 4 * NBLK * 16], F32)
            tmp4 = tmp_t[:, :].rearrange("p (c j k) -> p c j k", c=4, k=16)
            e4 = e_t[:, :].rearrange("p (c j k) -> p c j k", c=1, k=16)
            e4b = broadcast_tensor_aps(tmp4, e4)[1]
            cbj = cb64_t[:, :].rearrange("p (c j k) -> p c j k", c=4, k=16)
            cbjb = broadcast_tensor_aps(tmp4, cbj)[1]
            nc.vector.tensor_tensor(tmp4, e4b, cbjb, op=ALU.mult)
            raw_t = cpool.tile([128, 4 * NBLK], F32)
            nc.vector.tensor_reduce(
                raw_t[:, :], tmp_t[:, :].rearrange("p (cj k) -> p cj k", k=16), AX.X, op=ALU.add
            )
            # w4 = raw * (1/s), with 1/s broadcast over c (stride-0)
            w4_t = cpool.tile([128, 4 * NBLK], F32)
            w43 = w4_t[:, :].rearrange("p (c j) -> p c j", c=4)
            r43 = r_t[:, :].rearrange("p (c j) -> p c j", c=1)
            r43b = broadcast_tensor_aps(w43, r43)[1]
            nc.vector.tensor_tensor(
                w43, raw_t[:, :].rearrange("p (c j) -> p c j", c=4), r43b, op=ALU.mult
            )

            def wc(c, j):
                return w4_t[:, c * NBLK + j : c * NBLK + j + 1]

            jl = NBLK - 1
            u7_t = gpool.tile([128, BATCH], F16, tag="u7")
            v7_t = gpool.tile([128, BATCH], F16, tag="v7")

            o0h_t = [
                opool.tile([128, HB], F16, name=f"o0h{h}", tag=f"o0h{h}") for h in range(2)
            ]
            o_t = [None] + [
                opool.tile([128, BATCH], F16, name=f"o{j}", tag=f"o{j}")
                for j in range(1, NBLK)
            ]

            # ---- block 0 (two column-halves, earliest data) ----
            for h in range(2):
                a_ap = g0h_t[h][:, 0:HB]
                b_ap = g0h_t[h][:, HB:BATCH]
                u_t = wpool.tile([128, HB], F16, name=f"u0h{h}", tag="u0h")
                v_t = wpool.tile([128, HB], F16, name=f"v0h{h}", tag="v0h")
                t_t = wpool.tile([128, HB], F16, name=f"t0h{h}", tag="t0h")
                nc.scalar.activation(u_t[:, :], a_ap, AF.Identity, bias=wc(2, 0), scale=wc(3, 0))
                nc.vector.tensor_scalar(
                    v_t[:, :], a_ap, wc(1, 0), wc(0, 0), op0=ALU.mult, op1=ALU.add
                )
                nc.vector.tensor_tensor(t_t[:, :], u_t[:, :], b_ap, op=ALU.mult)
                nc.vector.tensor_tensor(o0h_t[h][:, :], t_t[:, :], v_t[:, :], op=ALU.add)

            # ---- blocks 1..6: streaming compute ----
            for j in range(1, NBLK - 1):
                a_ap = g_t[j - 1][:, 0:BATCH]
                b_ap = g_t[j - 1][:, BATCH : 2 * BATCH]
                u_t = wpool.tile([128, BATCH], F16, tag="u")
                v_t = wpool.tile([128, BATCH], F16, tag="v")
                t_t = wpool.tile([128, BATCH], F16, tag="t")
                nc.scalar.activation(u_t[:, :], a_ap, AF.Identity, bias=wc(2, j), scale=wc(3, j))
                nc.vector.tensor_scalar(
                    v_t[:, :], a_ap, wc(1, j), wc(0, j), op0=ALU.mult, op1=ALU.add
                )
                nc.vector.tensor_tensor(t_t[:, :], u_t[:, :], b_ap, op=ALU.mult)
                nc.vector.tensor_tensor(o_t[j][:, :], t_t[:, :], v_t[:, :], op=ALU.add)
                if j == 3:
                    # block 7 affine prep in mid-stream slack (a7 landed by now)
                    nc.scalar.activation(
                        u7_t[:, :], ga7_t[:, :], AF.Identity, bias=wc(2, jl), scale=wc(3, jl)
                    )
                    nc.vector.tensor_scalar(
                        v7_t[:, :], ga7_t[:, :], wc(1, jl), wc(0, jl), op0=ALU.mult, op1=ALU.add
                    )

            # ---- block 7 tail: only t+o remain after b7 (last load) lands ----
            t7_t = wpool.tile([128, BATCH], F16, tag="t7")
            nc.vector.tensor_tensor(t7_t[:, :], u7_t[:, :], gb7_t[:, :], op=ALU.mult)
            nc.vector.tensor_tensor(o_t[jl][:, :], t7_t[:, :], v7_t[:, :], op=ALU.add)

            # ---- stores: SAME sync ring, queued behind all loads (FIFO) ----
            for h in range(2):
                nc.sync.dma_start(y0h[h][:, :], o0h_t[h][:, :])
            for j in range(1, NBLK):
                nc.sync.dma_start(ys[j - 1][:, :], o_t[j][:, :])

    nc.compile()
    return nc


def _get_program():
    global _PROGRAM
    if _PROGRAM is None:
        _PROGRAM = _build_program()
    return _PROGRAM


def make_in_maps(x, weights, indices_a, indices_b):
    x = np.asarray(x, dtype=np.float32)
    w = np.asarray(weights, dtype=np.float32)
    ia = np.asarray(indices_a).astype(np.int64)
    ib = np.asarray(indices_b).astype(np.int64)

    xt16 = np.ascontiguousarray(x.T.astype(np.float16))  # (IN_DIM, BATCH)

    cb64 = np.ascontiguousarray(
        np.broadcast_to(_C.T.reshape(1, 64), (128, 64)), dtype=np.float32
    )

    jl = NBLK - 1
    in_maps = []
    for c in range(N_CORES):
        sl = slice(c * OPC, (c + 1) * OPC)
        ia_c = ia[sl].reshape(NBLK, 128)
        ib_c = ib[sl].reshape(NBLK, 128)
        wsh = w[sl]  # (OPC, 16)
        m = {
            "cb64": cb64,
            "wpre": np.ascontiguousarray(
                wsh.reshape(NBLK, 128, 16).transpose(1, 0, 2).reshape(128, NBLK * 16)
            ),
        }
        a0, b0 = xt16[ia_c[0]], xt16[ib_c[0]]
        for h in range(2):
            half = np.empty((128, 2, HB), dtype=np.float16)
            half[:, 0, :] = a0[:, h * HB : (h + 1) * HB]
            half[:, 1, :] = b0[:, h * HB : (h + 1) * HB]
            m[f"g0h{h}"] = np.ascontiguousarray(half.reshape(128, BATCH))
        for j in range(1, NBLK - 1):
            blk = np.empty((128, 2, BATCH), dtype=np.float16)
            blk[:, 0, :] = xt16[ia_c[j]]
            blk[:, 1, :] = xt16[ib_c[j]]
            m[f"g{j}"] = np.ascontiguousarray(blk.reshape(128, 2 * BATCH))
        m["ga7"] = np.ascontiguousarray(xt16[ia_c[jl]])
        m["gb7"] = np.ascontiguousarray(xt16[ib_c[jl]])
        in_maps.append(m)
    return in_maps


def run(inputs, trace=False):
    if trace:
        try:
            from antenv.axon_hooks import get_axon_ntff_profile_hook  # noqa: F401
        except ImportError:
            trace = False
    nc = _get_program()
    in_maps = make_in_maps(
        inputs["x"], inputs["weights"], inputs["indices_a"], inputs["indices_b"]
    )
    res = run_bass_kernel_spmd(nc, in_maps, core_ids=list(range(N_CORES)), trace=trace)
    outT = np.empty((OUT_DIM, BATCH), dtype=np.float32)
    for c in range(N_CORES):
        r = res.results[c]
        base = c * OPC
        y0 = np.concatenate([r["y0h0"], r["y0h1"]], axis=1).astype(np.float32)
        outT[base : base + 128] = y0
        for j in range(1, NBLK):
            outT[base + j * 128 : base + (j + 1) * 128] = r[f"y{j}"].astype(np.float32)
    return np.ascontiguousarray(outT.T), res


def kernel(**inputs):
    out, _ = run(inputs, trace=bool(os.environ.get("DL_TRACE")))
    return out


if __name__ == "__main__":
    rng = np.random.default_rng(0)
    inputs = {
        "x": rng.random((BATCH, IN_DIM), dtype=np.float32),
        "weights": rng.standard_normal((OUT_DIM, 16)).astype(np.float32),
        "indices_a": rng.integers(0, IN_DIM, size=OUT_DIM),
        "indices_b": rng.integers(0, IN_DIM, size=OUT_DIM),
    }
    out = kernel(**inputs)
    print(out.shape, out.dtype)



# revision 2
# speedup vs baseline: 1.0355x; 1.0355x over previous
"""DiffLogicLayer Trainium2 kernel — host-gather + u8-quantized 'a' operand with
weight-sensitivity routing, fp16 streaming, single sync-HWDGE DMA ring.

Math: for output neuron o with a = x[:, ia[o]], b = x[:, ib[o]], the 16 relaxed
gates are linear in {1, a, b, ab}:  out = W0 + W1*a + W2*b + W3*ab, where
W = softmax(weights) @ C. The softmax+C-fold is a (8192,16)->(8192,4) weight
preprocessing, done host-side in fp64 (16KB/core of constants shipped).

Per block (128 neurons x 2048 batch): u = W3*a + W2 (ScalarE), v = W1*a + W0,
t = u*b (DVE TT 2x), o = t + v (DVE TT 2x). Three block flavors per core:

- F (fp16 'a'): baseline path. u on ACT, v on DVE tensor_scalar (4x).
- A (u8 'a', ACT-heavy): ScalarE reads the u8 bytes directly (ACTIVATE is
  1x rate, dtype-independent) for BOTH u and v with /256 folded into the
  scale; DVE does only t, o.
- D (u8 'a', DVE-heavy): DVE unpacks byte pairs via uint16 bitwise ops into
  fp16 y = 256 + q/4 by OR-ing the byte into the mantissa of 0x5C00 (one
  tensor_scalar per half: and+or / shr+or, ~507ns each); v = TS(y) at 4x,
  u = ACT(y) contiguous; constants absorb the 256+q/4 -> q/256 affine map.
  Host packs D-block bytes interleaved (byte 2j = elem j, 2j+1 = elem H+j)
  so y comes out in natural batch order — no strided reads anywhere.

u8 quantization of 'a' costs max rel err 1.9e-2 if applied everywhere (gate:
2e-2). Routing fix: neurons are globally sorted by the weights-only
sensitivity bound S = max_b |W1 + W3*b| = max(|W1|, |W1+W3|); the top 1024
(12.5%) go to each core's single F block, the rest to u8 blocks. Simulated
max rel err: 5.8e-3. The neuron->position permutation is undone on host.

Sharding: tensor-parallel over out_dim (1024 neurons/core, host-chosen
permutation). ALL loads+stores on the sync HWDGE ring (FIFO; reads ~377GB/s
in-queue; mixed rings measured slower — v3/v14 lessons). GPSIMD unused (v5).
Full-tile contiguous DVE APs keep 2x/4x perf modes (v7). Per-core DMA:
6.46MB loads + 4.19MB stores vs 12.7MB in the all-fp16 version.

Output fp16; host concatenates, inverse-permutes, transposes, casts fp32.
"""

import os
import sys

import numpy as np

sys.path.insert(0, "/opt/trn_rl_repo")

import concourse.bacc as bacc
import concourse.mybir as mybir
from concourse import tile
from concourse.bass_utils import run_bass_kernel_spmd

AF = mybir.ActivationFunctionType
ALU = mybir.AluOpType
F32 = mybir.dt.float32
F16 = mybir.dt.float16
U8 = mybir.dt.uint8
U16 = mybir.dt.uint16

IN_DIM = 8192
OUT_DIM = 8192
BATCH = 2048
N_CORES = 8
OPC = OUT_DIM // N_CORES  # 1024 neurons per core
NBLK = OPC // 128  # 8 partition blocks per core
H = BATCH // 2

# Block flavor per in-core block index: D = u8/DVE-heavy, A = u8/ACT-heavy,
# F = fp16 (sensitive neurons). Block 0 is D so DVE starts as soon as the
# first u8 tile lands; block 7 is A so only t+o remain after the last load.
BLOCK_KIND = ["D", "F", "A", "D", "A", "D", "A", "A"]
FBLK = BLOCK_KIND.index("F")

# gate_k = C[k,0] + C[k,1]*a + C[k,2]*b + C[k,3]*ab  (difflogic convention)
_C = np.array(
    [
        [0, 0, 0, 0], [0, 0, 0, 1], [0, 1, 0, -1], [0, 1, 0, 0],
        [0, 0, 1, -1], [0, 0, 1, 0], [0, 1, 1, -2], [0, 1, 1, -1],
        [1, -1, -1, 1], [1, -1, -1, 2], [1, 0, -1, 0], [1, 0, -1, 1],
        [1, -1, 0, 0], [1, -1, 0, 1], [1, 0, 0, -1], [1, 0, 0, 0],
    ],
    dtype=np.float64,
)

_PROGRAM = None


def _build_program():
    nc = bacc.Bacc("TRN2", target_bir_lowering=False, debug=False)

    # cs[:, 4j:4j+4] = per-partition (su, bu, sv, bv) for block j
    cs = nc.dram_tensor("cs", (128, 4 * NBLK), F32, kind="ExternalInput")
    # u8 'a' tiles, pair-coalesced: a03 covers D0 (+A2, D3 via a23), see order
    a0 = nc.dram_tensor("a0", (128, BATCH), U8, kind="ExternalInput")
    a1f = nc.dram_tensor("a1f", (128, BATCH), F16, kind="ExternalInput")
    a23 = nc.dram_tensor("a23", (128, 2 * BATCH), U8, kind="ExternalInput")
    a45 = nc.dram_tensor("a45", (128, 2 * BATCH), U8, kind="ExternalInput")
    a67 = nc.dram_tensor("a67", (128, 2 * BATCH), U8, kind="ExternalInput")
    bts = [
        nc.dram_tensor(f"b{j}{j + 1}", (128, 2 * BATCH), F16, kind="ExternalInput")
        for j in (0, 2, 4, 6)
    ]
    ys = [
        nc.dram_tensor(f"y{j}", (128, BATCH), F16, kind="ExternalOutput")
        for j in range(NBLK)
    ]

    with tile.TileContext(nc) as tc:
        with (
            tc.tile_pool(name="const", bufs=1) as cpool,
            tc.tile_pool(name="gath", bufs=1) as gpool,
            tc.tile_pool(name="work", bufs=3) as wpool,
            tc.tile_pool(name="outp", bufs=1) as opool,
        ):
            # ---- loads: all on the sync HWDGE ring, in stream order ----
            cs_t = cpool.tile([128, 4 * NBLK], F32)
            nc.sync.dma_start(cs_t[:, :], cs[:, :])
            a0_t = gpool.tile([128, BATCH], U8, tag="a0")
            nc.sync.dma_start(a0_t[:, :], a0[:, :])
            b01_t = gpool.tile([128, 2 * BATCH], F16, tag="b01")
            nc.sync.dma_start(b01_t[:, :], bts[0][:, :])
            a1f_t = gpool.tile([128, BATCH], F16, tag="a1f")
            nc.sync.dma_start(a1f_t[:, :], a1f[:, :])
            a23_t = gpool.tile([128, 2 * BATCH], U8, tag="a23")
            nc.sync.dma_start(a23_t[:, :], a23[:, :])
            b23_t = gpool.tile([128, 2 * BATCH], F16, tag="b23")
            nc.sync.dma_start(b23_t[:, :], bts[1][:, :])
            a45_t = gpool.tile([128, 2 * BATCH], U8, tag="a45")
            nc.sync.dma_start(a45_t[:, :], a45[:, :])
            a67_t = gpool.tile([128, 2 * BATCH], U8, tag="a67")
            nc.sync.dma_start(a67_t[:, :], a67[:, :])
            b45_t = gpool.tile([128, 2 * BATCH], F16, tag="b45")
            nc.sync.dma_start(b45_t[:, :], bts[2][:, :])
            b67_t = gpool.tile([128, 2 * BATCH], F16, tag="b67")
            nc.sync.dma_start(b67_t[:, :], bts[3][:, :])

            def a8_ap(j):
                if j == 0:
                    return a0_t[:, :]
                src = {2: a23_t, 3: a23_t, 4: a45_t, 5: a45_t, 6: a67_t, 7: a67_t}[j]
                off = (j % 2) * BATCH
                return src[:, off : off + BATCH]

            def b_ap(j):
                src = bts_t[j // 2]
                off = (j % 2) * BATCH
                return src[:, off : off + BATCH]

            bts_t = [b01_t, b23_t, b45_t, b67_t]

            def su(j):
                return cs_t[:, 4 * j : 4 * j + 1]

            def bu(j):
                return cs_t[:, 4 * j + 1 : 4 * j + 2]

            def sv(j):
                return cs_t[:, 4 * j + 2 : 4 * j + 3]

            def bv(j):
                return cs_t[:, 4 * j + 3 : 4 * j + 4]

            o_t = [
                opool.tile([128, BATCH], F16, name=f"o{j}", tag=f"o{j}")
                for j in range(NBLK)
            ]

            for j in range(NBLK):
                kind = BLOCK_KIND[j]
                u_t = wpool.tile([128, BATCH], F16, name=f"u{j}", tag="u")
                v_t = wpool.tile([128, BATCH], F16, name=f"v{j}", tag="v")
                t_t = wpool.tile([128, BATCH], F16, name=f"t{j}", tag="t")
                if kind == "F":
                    a_ap = a1f_t[:, :]
                    nc.scalar.activation(
                        u_t[:, :], a_ap, AF.Identity, bias=bu(j), scale=su(j)
                    )
                    nc.vector.tensor_scalar(
                        v_t[:, :], a_ap, sv(j), bv(j), op0=ALU.mult, op1=ALU.add
                    )
                elif kind == "A":
                    a_ap = a8_ap(j)
                    nc.scalar.activation(
                        u_t[:, :], a_ap, AF.Identity, bias=bu(j), scale=su(j)
                    )
                    nc.scalar.activation(
                        v_t[:, :], a_ap, AF.Identity, bias=bv(j), scale=sv(j)
                    )
                else:  # D: unpack bytes to y = 256 + q/4, then v on DVE, u on ACT
                    w16 = a8_ap(j).bitcast(U16)
                    y_t = wpool.tile([128, BATCH], F16, name=f"y{j}d", tag="y")
                    nc.vector.tensor_scalar(
                        y_t[:, 0:H].bitcast(U16), w16, 0x00FF, 0x5C00,
                        op0=ALU.bitwise_and, op1=ALU.bitwise_or,
                    )
                    nc.vector.tensor_scalar(
                        y_t[:, H:BATCH].bitcast(U16), w16, 8, 0x5C00,
                        op0=ALU.logical_shift_right, op1=ALU.bitwise_or,
                    )
                    nc.scalar.activation(
                        u_t[:, :], y_t[:, :], AF.Identity, bias=bu(j), scale=su(j)
                    )
                    nc.vector.tensor_scalar(
                        v_t[:, :], y_t[:, :], sv(j), bv(j), op0=ALU.mult, op1=ALU.add
                    )
                nc.vector.tensor_tensor(t_t[:, :], u_t[:, :], b_ap(j), op=ALU.mult)
                nc.vector.tensor_tensor(o_t[j][:, :], t_t[:, :], v_t[:, :], op=ALU.add)

            # ---- stores: SAME sync ring, queued behind all loads (FIFO) ----
            for j in range(NBLK):
                nc.sync.dma_start(ys[j][:, :], o_t[j][:, :])

    nc.compile()
    return nc


def _get_program():
    global _PROGRAM
    if _PROGRAM is None:
        _PROGRAM = _build_program()
    return _PROGRAM


def make_in_maps(x, weights, indices_a, indices_b):
    x = np.asarray(x, dtype=np.float32)
    w = np.asarray(weights, dtype=np.float64)
    ia = np.asarray(indices_a).astype(np.int64)
    ib = np.asarray(indices_b).astype(np.int64)

    # softmax + C-fold (weight preprocessing, fp64)
    e = np.exp(w - w.max(axis=1, keepdims=True))
    sm = e / e.sum(axis=1, keepdims=True)
    W = sm @ _C  # (OUT_DIM, 4): W0..W3

    # global sensitivity routing: top-1024 neurons by S -> the F blocks
    S = np.maximum(np.abs(W[:, 1]), np.abs(W[:, 1] + W[:, 3]))
    order = np.argsort(-S, kind="stable")
    sens, rest = order[:1024], order[1024:]
    # nperm[core, blk, p] = original neuron id at that device position
    nperm = np.empty((N_CORES, NBLK, 128), dtype=np.int64)
    ri = 0
    for c in range(N_CORES):
        nperm[c, FBLK] = sens[c * 128 : (c + 1) * 128]
        for j in range(NBLK):
            if j == FBLK:
                continue
            nperm[c, j] = rest[ri : ri + 128]
            ri += 128

    xt16 = np.ascontiguousarray(x.T.astype(np.float16))  # (IN_DIM, BATCH)
    xt8 = np.clip(np.round(x.T * 256.0), 0, 255).astype(np.uint8)

    in_maps = []
    for c in range(N_CORES):
        cs = np.empty((128, 4 * NBLK), dtype=np.float32)
        m = {"cs": cs}
        ga8 = {}
        for j in range(NBLK):
            nid = nperm[c, j]
            W0, W1, W2, W3 = (W[nid, k] for k in range(4))
            kind = BLOCK_KIND[j]
            if kind == "F":
                su, bu, sv, bv = W3, W2, W1, W0
                m["a1f"] = np.ascontiguousarray(xt16[ia[nid]])
            elif kind == "A":
                su, bu, sv, bv = W3 / 256.0, W2, W1 / 256.0, W0
                ga8[j] = xt8[ia[nid]]
            else:  # D: y = 256 + q/4 -> q = 4*(y-256)
                su, bu = W3 / 64.0, W2 - 4.0 * W3
                sv, bv = W1 / 64.0, W0 - 4.0 * W1
                q = xt8[ia[nid]]
                il = np.empty((128, BATCH), dtype=np.uint8)
                il[:, 0::2] = q[:, :H]
                il[:, 1::2] = q[:, H:]
                ga8[j] = il
            cs[:, 4 * j + 0] = su
            cs[:, 4 * j + 1] = bu
            cs[:, 4 * j + 2] = sv
            cs[:, 4 * j + 3] = bv
        m["a0"] = np.ascontiguousarray(ga8[0])
        for pair in ((2, 3), (4, 5), (6, 7)):
            m[f"a{pair[0]}{pair[1]}"] = np.ascontiguousarray(
                np.concatenate([ga8[pair[0]], ga8[pair[1]]], axis=1)
            )
        for j in (0, 2, 4, 6):
            blk = np.empty((128, 2, BATCH), dtype=np.float16)
            blk[:, 0, :] = xt16[ib[nperm[c, j]]]
            blk[:, 1, :] = xt16[ib[nperm[c, j + 1]]]
            m[f"b{j}{j + 1}"] = np.ascontiguousarray(blk.reshape(128, 2 * BATCH))
        in_maps.append(m)
    return in_maps, nperm


def run(inputs, trace=False):
    if trace:
        try:
            from antenv.axon_hooks import get_axon_ntff_profile_hook  # noqa: F401
        except ImportError:
            trace = False
    nc = _get_program()
    in_maps, nperm = make_in_maps(
        inputs["x"], inputs["weights"], inputs["indices_a"], inputs["indices_b"]
    )
    res = run_bass_kernel_spmd(nc, in_maps, core_ids=list(range(N_CORES)), trace=trace)
    outT = np.empty((OUT_DIM, BATCH), dtype=np.float32)
    for c in range(N_CORES):
        r = res.results[c]
        for j in range(NBLK):
            outT[nperm[c, j]] = r[f"y{j}"].astype(np.float32)
    return np.ascontiguousarray(outT.T), res


def kernel(**inputs):
    out, _ = run(inputs, trace=bool(os.environ.get("DL_TRACE")))
    return out


if __name__ == "__main__":
    rng = np.random.default_rng(0)
    inputs = {
        "x": rng.random((BATCH, IN_DIM), dtype=np.float32),
        "weights": rng.standard_normal((OUT_DIM, 16)).astype(np.float32),
        "indices_a": rng.integers(0, IN_DIM, size=OUT_DIM),
        "indices_b": rng.integers(0, IN_DIM, size=OUT_DIM),
    }
    out = kernel(**inputs)
    print(out.shape, out.dtype)


# revision 3
# speedup vs baseline: 1.0533x; 1.0172x over previous
"""DiffLogicLayer Trainium2 kernel — host-gather + u8-quantized 'a' operand with
weight-sensitivity routing, fp16 streaming, pair-fused TT ops, single sync ring.

Math: out = W0 + W1*a + W2*b + W3*ab per neuron, W = softmax(weights) @ C
(the 16 difflogic gates are linear in {1, a, b, ab}). softmax+C-fold is done
host-side in fp64 (weight preprocessing; 16KB/core of constants shipped).

Per block (128 neurons x 2048 batch): u = W3*a + W2, v = W1*a + W0,
t = u*b, o = t + v. Block flavors:

- F (fp16 'a'): u on ACT, v on DVE tensor_scalar (4x).
- A (u8 'a'): ScalarE reads u8 directly (ACTIVATE is 1x, dtype-independent)
  for BOTH u and v, /256 folded into scale; DVE does only t, o.
- D (u8 'a'): DVE unpacks byte pairs via uint16 bitwise ops into fp16
  y = 256 + q/4 (OR the byte into the mantissa of 0x5C00; and+or / shr+or,
  one tensor_scalar per half, ~418ns each); v = TS(y) 4x, u = ACT(y)
  contiguous; constants absorb the affine remap. Host packs D-block bytes
  interleaved (byte 2j = elem j, 2j+1 = elem H+j) so y lands in natural
  batch order — no strided reads anywhere.

t and o run as PAIR-FUSED TTs over adjacent blocks (u/v/b/t/o live in
(128, 4096) pair tiles): TT carries no per-partition scalars so fusing is
legal, saving the 58-cycle + DRAIN overhead per op; stores become 4x 1MB.

u8 'a' everywhere costs max rel err 1.9e-2 (gate 2e-2). Fix: neurons are
globally sorted by the weights-only sensitivity S = max(|W1|, |W1+W3|); the
top 1024 (12.5%) go to each core's single F block. Simulated: 5.8e-3.

Sharding: tensor-parallel over out_dim, host-chosen neuron permutation
(undone on host). ALL loads+stores on the sync HWDGE ring (FIFO; ~400GB/s
in-queue; split rings measured slower — v3/v14). GPSIMD unused (v5).
Contiguous full-tile DVE APs keep 2x/4x modes (v7). Per-core DMA: 6.3MB
loads + 4.2MB stores (vs 12.7MB all-fp16).
"""

import os
import sys

import numpy as np

sys.path.insert(0, "/opt/trn_rl_repo")

import concourse.bacc as bacc
import concourse.mybir as mybir
from concourse import tile
from concourse.bass_utils import run_bass_kernel_spmd

AF = mybir.ActivationFunctionType
ALU = mybir.AluOpType
F32 = mybir.dt.float32
F16 = mybir.dt.float16
U8 = mybir.dt.uint8
U16 = mybir.dt.uint16

IN_DIM = 8192
OUT_DIM = 8192
BATCH = 2048
N_CORES = 8
OPC = OUT_DIM // N_CORES
NBLK = OPC // 128
H = BATCH // 2

# Block flavors. Blocks 0-2 are D so DVE starts as soon as a01/a23 land;
# the last pair (6,7) is (D,A): after b67 lands only unpack6/v6/t67/o67
# remain on DVE and u7/v7 are two early-emittable ACT ops.
BLOCK_KIND = ["D", "D", "D", "A", "A", "F", "D", "A"]
FBLK = BLOCK_KIND.index("F")

_C = np.array(
    [
        [0, 0, 0, 0], [0, 0, 0, 1], [0, 1, 0, -1], [0, 1, 0, 0],
        [0, 0, 1, -1], [0, 0, 1, 0], [0, 1, 1, -2], [0, 1, 1, -1],
        [1, -1, -1, 1], [1, -1, -1, 2], [1, 0, -1, 0], [1, 0, -1, 1],
        [1, -1, 0, 0], [1, -1, 0, 1], [1, 0, 0, -1], [1, 0, 0, 0],
    ],
    dtype=np.float64,
)

_PROGRAM = None


def _build_program():
    nc = bacc.Bacc("TRN2", target_bir_lowering=False, debug=False)

    cs = nc.dram_tensor("cs", (128, 4 * NBLK), F32, kind="ExternalInput")
    a01 = nc.dram_tensor("a01", (128, 2 * BATCH), U8, kind="ExternalInput")
    a23 = nc.dram_tensor("a23", (128, 2 * BATCH), U8, kind="ExternalInput")
    a4 = nc.dram_tensor("a4", (128, BATCH), U8, kind="ExternalInput")
    a5f = nc.dram_tensor("a5f", (128, BATCH), F16, kind="ExternalInput")
    a67 = nc.dram_tensor("a67", (128, 2 * BATCH), U8, kind="ExternalInput")
    bts = [
        nc.dram_tensor(f"b{p}{p + 1}", (128, 2 * BATCH), F16, kind="ExternalInput")
        for p in (0, 2, 4, 6)
    ]
    yts = [
        nc.dram_tensor(f"y{p}{p + 1}", (128, 2 * BATCH), F16, kind="ExternalOutput")
        for p in (0, 2, 4, 6)
    ]

    with tile.TileContext(nc) as tc:
        with (
            tc.tile_pool(name="const", bufs=1) as cpool,
            tc.tile_pool(name="gath", bufs=1) as gpool,
            tc.tile_pool(name="work", bufs=2) as wpool,
            tc.tile_pool(name="outp", bufs=1) as opool,
        ):
            # ---- loads: all on the sync HWDGE ring, in stream order ----
            cs_t = cpool.tile([128, 4 * NBLK], F32)
            nc.sync.dma_start(cs_t[:, :], cs[:, :])
            a01_t = gpool.tile([128, 2 * BATCH], U8, tag="a01")
            nc.sync.dma_start(a01_t[:, :], a01[:, :])
            b_t = [None] * 4
            b_t[0] = gpool.tile([128, 2 * BATCH], F16, name="b01t", tag="b01")
            nc.sync.dma_start(b_t[0][:, :], bts[0][:, :])
            a23_t = gpool.tile([128, 2 * BATCH], U8, tag="a23")
            nc.sync.dma_start(a23_t[:, :], a23[:, :])
            b_t[1] = gpool.tile([128, 2 * BATCH], F16, name="b23t", tag="b23")
            nc.sync.dma_start(b_t[1][:, :], bts[1][:, :])
            a4_t = gpool.tile([128, BATCH], U8, tag="a4")
            nc.sync.dma_start(a4_t[:, :], a4[:, :])
            a5f_t = gpool.tile([128, BATCH], F16, tag="a5f")
            nc.sync.dma_start(a5f_t[:, :], a5f[:, :])
            b_t[2] = gpool.tile([128, 2 * BATCH], F16, name="b45t", tag="b45")
            nc.sync.dma_start(b_t[2][:, :], bts[2][:, :])
            a67_t = gpool.tile([128, 2 * BATCH], U8, tag="a67")
            nc.sync.dma_start(a67_t[:, :], a67[:, :])
            b_t[3] = gpool.tile([128, 2 * BATCH], F16, name="b67t", tag="b67")
            nc.sync.dma_start(b_t[3][:, :], bts[3][:, :])

            def a8_ap(j):
                src = {0: a01_t, 1: a01_t, 2: a23_t, 3: a23_t,
                       4: a4_t, 6: a67_t, 7: a67_t}[j]
                if j == 4:
                    return src[:, :]
                off = (j % 2) * BATCH
                return src[:, off : off + BATCH]

            def su(j):
                return cs_t[:, 4 * j : 4 * j + 1]

            def bu(j):
                return cs_t[:, 4 * j + 1 : 4 * j + 2]

            def sv(j):
                return cs_t[:, 4 * j + 2 : 4 * j + 3]

            def bv(j):
                return cs_t[:, 4 * j + 3 : 4 * j + 4]

            o_t = [
                opool.tile([128, 2 * BATCH], F16, name=f"o{p}{p + 1}", tag=f"o{p}")
                for p in (0, 2, 4, 6)
            ]

            for pi in range(4):
                u_t = wpool.tile([128, 2 * BATCH], F16, name=f"u{pi}", tag="u")
                v_t = wpool.tile([128, 2 * BATCH], F16, name=f"v{pi}", tag="v")
                t_t = wpool.tile([128, 2 * BATCH], F16, name=f"t{pi}", tag="t")
                for half, j in enumerate((2 * pi, 2 * pi + 1)):
                    kind = BLOCK_KIND[j]
                    lo, hi = half * BATCH, (half + 1) * BATCH
                    u_ap = u_t[:, lo:hi]
                    v_ap = v_t[:, lo:hi]
                    if kind == "F":
                        a_ap = a5f_t[:, :]
                        nc.scalar.activation(
                            u_ap, a_ap, AF.Identity, bias=bu(j), scale=su(j)
                        )
                        nc.vector.tensor_scalar(
                            v_ap, a_ap, sv(j), bv(j), op0=ALU.mult, op1=ALU.add
                        )
                    elif kind == "A":
                        a_ap = a8_ap(j)
                        nc.scalar.activation(
                            u_ap, a_ap, AF.Identity, bias=bu(j), scale=su(j)
                        )
                        nc.scalar.activation(
                            v_ap, a_ap, AF.Identity, bias=bv(j), scale=sv(j)
                        )
                    else:  # D
                        w16 = a8_ap(j).bitcast(U16)
                        y_t = wpool.tile([128, BATCH], F16, name=f"y{j}d", tag="y")
                        nc.vector.tensor_scalar(
                            y_t[:, 0:H].bitcast(U16), w16, 0x00FF, 0x5C00,
                            op0=ALU.bitwise_and, op1=ALU.bitwise_or,
                        )
                        nc.vector.tensor_scalar(
                            y_t[:, H:BATCH].bitcast(U16), w16, 8, 0x5C00,
                            op0=ALU.logical_shift_right, op1=ALU.bitwise_or,
                        )
                        nc.scalar.activation(
                            u_ap, y_t[:, :], AF.Identity, bias=bu(j), scale=su(j)
                        )
                        nc.vector.tensor_scalar(
                            v_ap, y_t[:, :], sv(j), bv(j), op0=ALU.mult, op1=ALU.add
                        )
                # pair-fused t and o
                nc.vector.tensor_tensor(
                    t_t[:, :], u_t[:, :], b_t[pi][:, :], op=ALU.mult
                )
                nc.vector.tensor_tensor(
                    o_t[pi][:, :], t_t[:, :], v_t[:, :], op=ALU.add
                )

            # ---- stores: SAME sync ring, queued behind all loads (FIFO) ----
            for pi in range(4):
                nc.sync.dma_start(yts[pi][:, :], o_t[pi][:, :])

    nc.compile()
    return nc


def _get_program():
    global _PROGRAM
    if _PROGRAM is None:
        _PROGRAM = _build_program()
    return _PROGRAM


def make_in_maps(x, weights, indices_a, indices_b):
    x = np.asarray(x, dtype=np.float32)
    w = np.asarray(weights, dtype=np.float64)
    ia = np.asarray(indices_a).astype(np.int64)
    ib = np.asarray(indices_b).astype(np.int64)

    e = np.exp(w - w.max(axis=1, keepdims=True))
    sm = e / e.sum(axis=1, keepdims=True)
    W = sm @ _C  # (OUT_DIM, 4): W0..W3

    S = np.maximum(np.abs(W[:, 1]), np.abs(W[:, 1] + W[:, 3]))
    order = np.argsort(-S, kind="stable")
    sens, rest = order[:1024], order[1024:]
    nperm = np.empty((N_CORES, NBLK, 128), dtype=np.int64)
    ri = 0
    for c in range(N_CORES):
        nperm[c, FBLK] = sens[c * 128 : (c + 1) * 128]
        for j in range(NBLK):
            if j == FBLK:
                continue
            nperm[c, j] = rest[ri : ri + 128]
            ri += 128

    xt16 = np.ascontiguousarray(x.T.astype(np.float16))  # (IN_DIM, BATCH)
    xt8 = np.clip(np.round(x.T * np.float32(256.0)), 0, 255).astype(np.uint8)

    in_maps = []
    for c in range(N_CORES):
        cs = np.empty((128, 4 * NBLK), dtype=np.float32)
        m = {"cs": cs}
        ga8 = {}
        for j in range(NBLK):
            nid = nperm[c, j]
            W0, W1, W2, W3 = (W[nid, k] for k in range(4))
            kind = BLOCK_KIND[j]
            if kind == "F":
                su, bu, sv, bv = W3, W2, W1, W0
                m["a5f"] = np.ascontiguousarray(xt16[ia[nid]])
            elif kind == "A":
                su, bu, sv, bv = W3 / 256.0, W2, W1 / 256.0, W0
                ga8[j] = xt8[ia[nid]]
            else:  # D: y = 256 + q/4 -> q = 4*(y-256)
                su, bu = W3 / 64.0, W2 - 4.0 * W3
                sv, bv = W1 / 64.0, W0 - 4.0 * W1
                q = xt8[ia[nid]]
                il = np.empty((128, BATCH), dtype=np.uint8)
                il[:, 0::2] = q[:, :H]
                il[:, 1::2] = q[:, H:]
                ga8[j] = il
            cs[:, 4 * j + 0] = su
            cs[:, 4 * j + 1] = bu
            cs[:, 4 * j + 2] = sv
            cs[:, 4 * j + 3] = bv
        m["a4"] = np.ascontiguousarray(ga8[4])
        for pair in ((0, 1), (2, 3), (6, 7)):
            m[f"a{pair[0]}{pair[1]}"] = np.ascontiguousarray(
                np.concatenate([ga8[pair[0]], ga8[pair[1]]], axis=1)
            )
        for p in (0, 2, 4, 6):
            blk = np.empty((128, 2, BATCH), dtype=np.float16)
            blk[:, 0, :] = xt16[ib[nperm[c, p]]]
            blk[:, 1, :] = xt16[ib[nperm[c, p + 1]]]
            m[f"b{p}{p + 1}"] = np.ascontiguousarray(blk.reshape(128, 2 * BATCH))
        in_maps.append(m)
    return in_maps, nperm


def run(inputs, trace=False):
    if trace:
        try:
            from antenv.axon_hooks import get_axon_ntff_profile_hook  # noqa: F401
        except ImportError:
            trace = False
    nc = _get_program()
    in_maps, nperm = make_in_maps(
        inputs["x"], inputs["weights"], inputs["indices_a"], inputs["indices_b"]
    )
    res = run_bass_kernel_spmd(nc, in_maps, core_ids=list(range(N_CORES)), trace=trace)
    outT = np.empty((OUT_DIM, BATCH), dtype=np.float32)
    for c in range(N_CORES):
        r = res.results[c]
        for p in (0, 2, 4, 6):
            pair = r[f"y{p}{p + 1}"].astype(np.float32)
            outT[nperm[c, p]] = pair[:, :BATCH]
            outT[nperm[c, p + 1]] = pair[:, BATCH:]
    return np.ascontiguousarray(outT.T), res


def kernel(**inputs):
    out, _ = run(inputs, trace=bool(os.environ.get("DL_TRACE")))
    return out


if __name__ == "__main__":
    rng = np.random.default_rng(0)
    inputs = {
        "x": rng.random((BATCH, IN_DIM), dtype=np.float32),
        "weights": rng.standard_normal((OUT_DIM, 16)).astype(np.float32),
        "indices_a": rng.integers(0, IN_DIM, size=OUT_DIM),
        "indices_b": rng.integers(0, IN_DIM, size=OUT_DIM),
    }
    out = kernel(**inputs)
    print(out.shape, out.dtype)
